# revision 1
# baseline (speedup 1.0000x reference)
"""Trainium2 Bass kernel for nn_Interpolator: pilot-to-subcarrier linear
interpolation with learned per-subcarrier weights.

Math: out[b, t] = alpha[t] * Hp[b, right[t]] + beta[t] * Hp[b, left[t]]
where Hp = [H, extrapolated last column] and left/right come from a
searchsorted of subcarrier indices against (0-based) pilot positions.

The op is linear in H, so it collapses to out = H @ W with a sparse
W [256, 4096] built on the host from (pilot_loc, alpha, beta); the
extrapolation column folds into W's last two rows.

On-device this is a TensorE matmul in bf16 with error compensation:
H is split on the host into bf16 hi + lo parts (H = hi + lo to ~2^-18
relative), and out = hi@W + lo@W accumulates exactly in fp32 PSUM.
bf16 runs the PE at 1 cycle/row (4x the fp32 rate). If W itself is not
exactly representable in bf16, a third hi@W_lo term is added. Per
512-wide output chunk only the 64-aligned k-row windows that are
actually nonzero in W are contracted. Real/imag are interleaved on-chip
with stride-2 copies so the final [128, 8192] store is one fully
contiguous DMA per 128-row batch tile.

Sharding: data-parallel over the batch dim, 2048 rows per core x 8 cores.
"""

import os
import sys

if os.path.isdir("/opt/trn_rl_repo") and "/opt/trn_rl_repo" not in sys.path:
    sys.path.insert(0, "/opt/trn_rl_repo")

import ml_dtypes
import numpy as np

_BF16 = np.dtype(ml_dtypes.bfloat16)

_B, _P, _NFFT = 16384, 256, 4096
_NC = 8
_BS = _B // _NC          # rows per core
_PT = 128                # partition tile (batch rows per tile)
_NBT = _BS // _PT        # batch tiles per core
_CH = 512                # output-chunk width (one PSUM bank of fp32)
_NCHUNK = _NFFT // _CH

_cache = {}


def _interp_matrix(pilot_loc, alpha, beta):
    """W [256, 4096] f32 such that out = H @ W reproduces the reference."""
    p = pilot_loc.astype(np.float64) - 1.0  # reference: 1-based -> 0-based
    pp = np.concatenate([p, [float(_NFFT - 1)]])
    t = np.arange(_NFFT)
    left = np.clip(np.searchsorted(pp, t, side="right") - 1, 0, _P - 1)
    right = left + 1
    Wf = np.zeros((_P + 1, _NFFT), np.float64)
    Wf[left, t] += beta.astype(np.float64)
    Wf[right, t] += alpha.astype(np.float64)
    # Hp[:, P] = H[:, P-1] + slope * (NFFT-1 - p[-1]),
    # slope = (H[:, P-1] - H[:, P-2]) / (p[-1] - p[-2])  -> linear in H.
    d = (float(_NFFT - 1) - p[-1]) / (p[-1] - p[-2])
    W = Wf[:_P]
    W[_P - 1] += (1.0 + d) * Wf[_P]
    W[_P - 2] += (-d) * Wf[_P]
    return np.ascontiguousarray(W.astype(np.float32))


def _chunk_pieces(W):
    """Per 512-col chunk: which 128-row halves of W have any nonzeros.

    Each piece is (half, lo, hi) == (half, 0, 128): a full half-tile.
    Full K=128 slices keep every matmul at PE tile_position (0, 0) —
    mixing sub-128 tile_positions across accumulation groups crashes the
    device, and matmul cycle cost is K-independent anyway.
    """
    out = []
    for c in range(_NCHUNK):
        cols = W[:, c * _CH:(c + 1) * _CH]
        nz = np.nonzero(np.any(cols != 0.0, axis=1))[0]
        k_lo, k_hi = int(nz.min()), int(nz.max())
        pieces = []
        for half in (0, 1):
            if k_lo <= 128 * half + 127 and k_hi >= 128 * half:
                pieces.append((half, 0, 128))
        out.append(tuple(pieces))
    return tuple(out)


def _bf16_split(x):
    hi = x.astype(_BF16)
    lo = (x - hi.astype(np.float32)).astype(_BF16)
    return hi, lo


def _build_program(pieces_per_chunk, use_wlo, repeats=1,
                   bench_internal_out=False, out_quarters=True,
                   in_ring_scalar=True):
    from contextlib import ExitStack

    import concourse.bacc as bacc
    import concourse.bass as bass
    import concourse.mybir as mybir
    import concourse.tile as tile
    from concourse.masks import make_identity

    f32 = mybir.dt.float32
    bf16 = mybir.dt.bfloat16

    nc = bacc.Bacc("TRN2", target_bir_lowering=False, debug=False,
                   num_devices=_NC)
    # Packed input: columns [hrh | hrl | hih | hil], one DMA per tile.
    h_in = nc.dram_tensor("hx", [_BS, 4 * _P], bf16,
                          kind="ExternalInput").ap()
    w_in = {"h": nc.dram_tensor("wh", [_P, _NFFT], bf16,
                                kind="ExternalInput").ap()}
    if use_wlo:
        w_in["l"] = nc.dram_tensor("wl", [_P, _NFFT], bf16,
                                   kind="ExternalInput").ap()
    if bench_internal_out:
        # Benchmark mode: same DMA traffic, but keep the 64MB buffer
        # device-internal so PJRT only moves a tiny result per call.
        out = nc.dram_tensor("out", [_BS, 2 * _NFFT], f32).ap()
        done = nc.dram_tensor("done", [1, 4], f32,
                              kind="ExternalOutput").ap()
    else:
        out = nc.dram_tensor("out", [_BS, 2 * _NFFT], f32,
                             kind="ExternalOutput").ap()
        done = None

    with tile.TileContext(nc) as tc, ExitStack() as ctx:
        const_pool = ctx.enter_context(tc.tile_pool(name="const", bufs=1))
        in_pool = ctx.enter_context(tc.tile_pool(name="inp", bufs=3))
        ht_pool = ctx.enter_context(tc.tile_pool(name="ht", bufs=2))
        out_pool = ctx.enter_context(tc.tile_pool(name="outp", bufs=2))
        ps_t = ctx.enter_context(tc.tile_pool(name="pst", bufs=2,
                                              space="PSUM"))
        ps_mm = ctx.enter_context(tc.tile_pool(name="psm", bufs=4,
                                               space="PSUM"))

        ident = const_pool.tile([128, 128], bf16, tag="ident")
        make_identity(nc, ident[:])
        # W halves in SBUF: w_sb[part][half] is rows [128*half, 128*half+128)
        # Input/weight loads go on the scalar-engine HWDGE ring so they
        # overlap the output stores on the sync ring (per-ring FIFO).
        in_dma = nc.scalar if in_ring_scalar else nc.sync
        w_sb = {}
        for part, wap in w_in.items():
            for h in (0, 1):
                wt = const_pool.tile([128, _NFFT], bf16, tag=f"w{part}{h}")
                in_dma.dma_start(wt[:], wap[128 * h:128 * (h + 1), :])
                w_sb[(part, h)] = wt

        copy_idx = 0
        for bt in [b for _ in range(repeats) for b in range(_NBT)]:
            hx = in_pool.tile([128, 4 * _P], bf16, tag="hx")
            in_dma.dma_start(hx[:], h_in[bass.ts(bt, 128), :])

            hT = {}
            for j, name in enumerate(("hrh", "hrl", "hih", "hil")):
                for h in (0, 1):
                    pst = ps_t.tile([128, 128], bf16, tag="pst")
                    nc.tensor.transpose(
                        pst[:], hx[:, bass.ts(2 * j + h, 128)], ident[:])
                    sb = ht_pool.tile([128, 128], bf16, tag=f"hT_{name}{h}")
                    nc.vector.tensor_copy(sb[:], pst[:])
                    hT[(name, h)] = sb

            ot = out_pool.tile([128, 2 * _NFFT], f32, tag="ot")
            for c in range(_NCHUNK):
                pieces = pieces_per_chunk[c]
                terms = [("h", "h"), ("l", "h")]
                if use_wlo:
                    terms.append(("h", "l"))
                n_mm = len(pieces) * len(terms)
                for x, parity in (("r", 0), ("i", 1)):
                    ps = ps_mm.tile([128, _CH], f32, tag="ps")
                    j = 0
                    for (h, lo, hi_) in pieces:
                        for (hp, wp) in terms:
                            nc.tensor.matmul(
                                ps[:],
                                hT[(f"h{x}{hp}", h)][lo:hi_, :],
                                w_sb[(wp, h)][lo:hi_,
                                              c * _CH:(c + 1) * _CH],
                                start=(j == 0),
                                stop=(j == n_mm - 1),
                            )
                            j += 1
                    dst = ot[:, 2 * _CH * c + parity:2 * _CH * (c + 1):2]
                    # ~2:1 vector:scalar split keeps the two engines balanced
                    # (ACT copies are ~2x slower than DVE).
                    if copy_idx % 3 == 2:
                        nc.scalar.copy(dst, ps[:])
                    else:
                        nc.vector.tensor_copy(dst, ps[:])
                    copy_idx += 1
                if out_quarters and c % 2 == 1:
                    # store finished 1MB quarter; keeps the write ring fed
                    # early and shrinks the tail drain.
                    q = c // 2
                    nc.sync.dma_start(
                        out[bass.ts(bt, 128), bass.ts(q, 2 * _CH * 2)],
                        ot[:, bass.ts(q, 2 * _CH * 2)])
            if not out_quarters:
                nc.sync.dma_start(out[bass.ts(bt, 128), :], ot[:])

        if done is not None:
            dn = const_pool.tile([1, 4], f32, tag="done")
            nc.vector.tensor_copy(dn[:], ot[0:1, 0:4])
            nc.sync.dma_start(done[:], dn[:])

    nc.compile()
    return nc


def _get_program(pieces, use_wlo):
    key = (pieces, use_wlo)
    prog = _cache.get(key)
    if prog is None:
        prog = _build_program(pieces, use_wlo)
        _cache[key] = prog
    return prog


def kernel(H_real, H_imag, pilot_loc, alpha, beta):
    H_real = np.ascontiguousarray(np.asarray(H_real, dtype=np.float32))
    H_imag = np.ascontiguousarray(np.asarray(H_imag, dtype=np.float32))
    pilot_loc = np.asarray(pilot_loc, dtype=np.float32)
    alpha = np.asarray(alpha, dtype=np.float32)
    beta = np.asarray(beta, dtype=np.float32)

    W = _interp_matrix(pilot_loc, alpha, beta)
    w_hi, w_lo = _bf16_split(W)
    use_wlo = bool(np.any(np.asarray(w_lo) != 0))
    pieces = _chunk_pieces(W)
    nc = _get_program(pieces, use_wlo)

    hr_hi, hr_lo = _bf16_split(H_real)
    hi_hi, hi_lo = _bf16_split(H_imag)

    from concourse.bass_utils import run_bass_kernel_spmd

    hx = np.concatenate([hr_hi, hr_lo, hi_hi, hi_lo], axis=1)
    in_maps = []
    for i in range(_NC):
        m = {
            "hx": np.ascontiguousarray(hx[i * _BS:(i + 1) * _BS]),
            "wh": w_hi,
        }
        if use_wlo:
            m["wl"] = w_lo
        in_maps.append(m)
    res = run_bass_kernel_spmd(nc, in_maps, list(range(_NC))).results
    return np.concatenate(
        [r["out"].reshape(_BS, _NFFT, 2) for r in res], axis=0
    )



# revision 3
# speedup vs baseline: 1.5620x; 1.5620x over previous
"""Trainium2 Bass kernel for nn_Interpolator: pilot-to-subcarrier linear
interpolation with learned per-subcarrier weights.

Math: out[b, t] = alpha[t] * Hp[b, right[t]] + beta[t] * Hp[b, left[t]]
where Hp = [H, extrapolated last column] and left/right come from a
searchsorted of subcarrier indices against (0-based) pilot positions.

The op is linear in H, so it collapses to out = H @ W with a sparse
W [256, 4096] built on the host from (pilot_loc, alpha, beta); the
extrapolation column folds into W's last two rows.

On-device this is a TensorE matmul in bf16. The rel-err budget (2e-2)
is far above bf16 rounding (~1e-3), so H is sent as plain bf16 (no
error-compensation terms) and the output is stored as fp16 — this
kernel is DMA-bound, and fp16 halves the dominant store traffic. If W
is not exactly bf16-representable, a compensating hi@W_lo term is
added. Per 512-wide output chunk only the 128-row halves of W that are
nonzero are contracted (full-K slices keep every matmul at PE
tile_position (0,0) — mixing sub-128 tile_positions across
accumulation groups crashes the device, and matmul cycle cost is
K-independent anyway).

DRAM output layout is [BS, 8192] fp16 with real in cols [0:4096] and
imag in [4096:8192]; the host interleaves r/i and upcasts to f32 while
unsharding. This keeps the PSUM->SBUF drain copies fully contiguous.

Sharding: data-parallel over the batch dim, 2048 rows per core x 8 cores.
"""

import os
import sys

if os.path.isdir("/opt/trn_rl_repo") and "/opt/trn_rl_repo" not in sys.path:
    sys.path.insert(0, "/opt/trn_rl_repo")

import ml_dtypes
import numpy as np

_BF16 = np.dtype(ml_dtypes.bfloat16)

_B, _P, _NFFT = 16384, 256, 4096
_NC = 8
_BS = _B // _NC          # rows per core
_PT = 128                # partition tile (batch rows per tile)
_NBT = _BS // _PT        # batch tiles per core
_CH = 512                # output-chunk width (one PSUM bank of fp32)
_NCHUNK = _NFFT // _CH

_cache = {}


def _interp_matrix(pilot_loc, alpha, beta):
    """W [256, 4096] f32 such that out = H @ W reproduces the reference."""
    p = pilot_loc.astype(np.float64) - 1.0  # reference: 1-based -> 0-based
    pp = np.concatenate([p, [float(_NFFT - 1)]])
    t = np.arange(_NFFT)
    left = np.clip(np.searchsorted(pp, t, side="right") - 1, 0, _P - 1)
    right = left + 1
    Wf = np.zeros((_P + 1, _NFFT), np.float64)
    Wf[left, t] += beta.astype(np.float64)
    Wf[right, t] += alpha.astype(np.float64)
    # Hp[:, P] = H[:, P-1] + slope * (NFFT-1 - p[-1]),
    # slope = (H[:, P-1] - H[:, P-2]) / (p[-1] - p[-2])  -> linear in H.
    d = (float(_NFFT - 1) - p[-1]) / (p[-1] - p[-2])
    W = Wf[:_P]
    W[_P - 1] += (1.0 + d) * Wf[_P]
    W[_P - 2] += (-d) * Wf[_P]
    return np.ascontiguousarray(W.astype(np.float32))


def _chunk_pieces(W):
    """Per 512-col chunk: which 128-row halves of W have any nonzeros."""
    out = []
    for c in range(_NCHUNK):
        cols = W[:, c * _CH:(c + 1) * _CH]
        nz = np.nonzero(np.any(cols != 0.0, axis=1))[0]
        k_lo, k_hi = int(nz.min()), int(nz.max())
        pieces = []
        for half in (0, 1):
            if k_lo <= 128 * half + 127 and k_hi >= 128 * half:
                pieces.append(half)
        out.append(tuple(pieces))
    return tuple(out)


def _bf16_split(x):
    hi = x.astype(_BF16)
    lo = (x - hi.astype(np.float32)).astype(_BF16)
    return hi, lo


def _build_program(pieces_per_chunk, use_wlo, store_every=2,
                   copy_cycle="vvs", repeats=1):
    from contextlib import ExitStack

    import concourse.bacc as bacc
    import concourse.bass as bass
    import concourse.mybir as mybir
    import concourse.tile as tile
    from concourse.masks import make_identity

    f32 = mybir.dt.float32
    f16 = mybir.dt.float16
    bf16 = mybir.dt.bfloat16

    nc = bacc.Bacc("TRN2", target_bir_lowering=False, debug=False,
                   num_devices=_NC)
    # Packed input: columns [hrh | hih], one DMA per tile.
    h_in = nc.dram_tensor("hx", [_BS, 2 * _P], bf16,
                          kind="ExternalInput").ap()
    w_in = {"h": nc.dram_tensor("wh", [_P, _NFFT], bf16,
                                kind="ExternalInput").ap()}
    if use_wlo:
        w_in["l"] = nc.dram_tensor("wl", [_P, _NFFT], bf16,
                                   kind="ExternalInput").ap()
    # real block then imag block; host interleaves + upcasts.
    out = nc.dram_tensor("out", [_BS, 2 * _NFFT], f16,
                         kind="ExternalOutput").ap()

    with tile.TileContext(nc) as tc, ExitStack() as ctx:
        const_pool = ctx.enter_context(tc.tile_pool(name="const", bufs=1))
        in_pool = ctx.enter_context(tc.tile_pool(name="inp", bufs=3))
        ht_pool = ctx.enter_context(tc.tile_pool(name="ht", bufs=2))
        out_pool = ctx.enter_context(tc.tile_pool(name="outp", bufs=2))
        ps_t = ctx.enter_context(tc.tile_pool(name="pst", bufs=2,
                                              space="PSUM"))
        ps_mm = ctx.enter_context(tc.tile_pool(name="psm", bufs=4,
                                               space="PSUM"))

        ident = const_pool.tile([128, 128], bf16, tag="ident")
        make_identity(nc, ident[:])
        # Input/weight loads go on the scalar-engine HWDGE ring so they
        # overlap the output stores on the sync ring (per-ring FIFO).
        w_sb = {}
        for part, wap in w_in.items():
            for h in (0, 1):
                wt = const_pool.tile([128, _NFFT], bf16, tag=f"w{part}{h}")
                nc.scalar.dma_start(wt[:], wap[128 * h:128 * (h + 1), :])
                w_sb[(part, h)] = wt

        terms = [("h", "h")]
        if use_wlo:
            terms.append(("h", "l"))

        copy_idx = 0
        for bt in [b for _ in range(repeats) for b in range(_NBT)]:
            hx = in_pool.tile([128, 2 * _P], bf16, tag="hx")
            nc.scalar.dma_start(hx[:], h_in[bass.ts(bt, 128), :])

            hT = {}
            for j, name in enumerate(("hr", "hi")):
                for h in (0, 1):
                    pst = ps_t.tile([128, 128], bf16, tag="pst")
                    nc.tensor.transpose(
                        pst[:], hx[:, bass.ts(2 * j + h, 128)], ident[:])
                    sb = ht_pool.tile([128, 128], bf16, tag=f"hT_{name}{h}")
                    nc.vector.tensor_copy(sb[:], pst[:])
                    hT[(name, h)] = sb

            ot = out_pool.tile([128, 2 * _NFFT], f16, tag="ot")
            for c in range(_NCHUNK):
                pieces = pieces_per_chunk[c]
                n_mm = len(pieces) * len(terms)
                for x, parity in (("r", 0), ("i", 1)):
                    ps = ps_mm.tile([128, _CH], f32, tag="ps")
                    j = 0
                    for h in pieces:
                        for (hp, wp) in terms:
                            nc.tensor.matmul(
                                ps[:],
                                hT[(f"h{x}", h)][:],
                                w_sb[(wp, h)][:, c * _CH:(c + 1) * _CH],
                                start=(j == 0),
                                stop=(j == n_mm - 1),
                            )
                            j += 1
                    dst = ot[:, _NFFT * parity + _CH * c:
                             _NFFT * parity + _CH * (c + 1)]
                    # rotate PSUM-drain copies across DVE ('v') and ACT
                    # ('s') to keep both engines under the DMA time.
                    eng = copy_cycle[copy_idx % len(copy_cycle)]
                    if eng == "s":
                        nc.scalar.copy(dst, ps[:])
                    else:
                        nc.vector.tensor_copy(dst, ps[:])
                    copy_idx += 1
                if (c + 1) % store_every == 0:
                    # store finished slices early; keeps the write ring
                    # fed and shrinks the tail drain.
                    w0 = _CH * (c + 1 - store_every)
                    w1 = _CH * (c + 1)
                    for parity in (0, 1):
                        nc.sync.dma_start(
                            out[bass.ts(bt, 128),
                                _NFFT * parity + w0:_NFFT * parity + w1],
                            ot[:, _NFFT * parity + w0:
                               _NFFT * parity + w1])

    nc.compile()
    return nc


def _get_program(pieces, use_wlo):
    # experiment knobs (default values are the tuned ones)
    se = int(os.environ.get("K_STORE_EVERY", "2"))
    cc = os.environ.get("K_COPY_CYCLE", "vvs")
    key = (pieces, use_wlo, se, cc)
    prog = _cache.get(key)
    if prog is None:
        prog = _build_program(pieces, use_wlo, store_every=se,
                              copy_cycle=cc)
        _cache[key] = prog
    return prog


def _make_in_maps(H_real, H_imag, W):
    w_hi, w_lo = _bf16_split(W)
    use_wlo = bool(np.any(np.asarray(w_lo) != 0))
    hx = np.concatenate(
        [H_real.astype(_BF16), H_imag.astype(_BF16)], axis=1)
    in_maps = []
    for i in range(_NC):
        m = {
            "hx": np.ascontiguousarray(hx[i * _BS:(i + 1) * _BS]),
            "wh": w_hi,
        }
        if use_wlo:
            m["wl"] = w_lo
        in_maps.append(m)
    return in_maps, use_wlo


def kernel(H_real, H_imag, pilot_loc, alpha, beta):
    H_real = np.ascontiguousarray(np.asarray(H_real, dtype=np.float32))
    H_imag = np.ascontiguousarray(np.asarray(H_imag, dtype=np.float32))
    pilot_loc = np.asarray(pilot_loc, dtype=np.float32)
    alpha = np.asarray(alpha, dtype=np.float32)
    beta = np.asarray(beta, dtype=np.float32)

    W = _interp_matrix(pilot_loc, alpha, beta)
    in_maps, use_wlo = _make_in_maps(H_real, H_imag, W)
    nc = _get_program(_chunk_pieces(W), use_wlo)

    from concourse.bass_utils import run_bass_kernel_spmd

    res = run_bass_kernel_spmd(nc, in_maps, list(range(_NC))).results
    full = np.empty((_B, _NFFT, 2), dtype=np.float32)
    for i, r in enumerate(res):
        o = r["out"]
        full[i * _BS:(i + 1) * _BS, :, 0] = o[:, :_NFFT]
        full[i * _BS:(i + 1) * _BS, :, 1] = o[:, _NFFT:]
    return full


# revision 5
# speedup vs baseline: 1.8208x; 1.1657x over previous
"""Trainium2 Bass kernel for nn_Interpolator: pilot-to-subcarrier linear
interpolation with learned per-subcarrier weights.

Math: out[b, t] = alpha[t] * Hp[b, right[t]] + beta[t] * Hp[b, left[t]]
where Hp = [H, extrapolated last column] and left/right come from a
searchsorted of subcarrier indices against (0-based) pilot positions.

The op is linear in H, so it collapses to out = H @ W with a sparse
W [256, 4096] built on the host from (pilot_loc, alpha, beta); the
extrapolation column folds into W's last two rows.

On-device this is a TensorE matmul in bf16. The rel-err budget (2e-2)
is far above bf16 rounding (~1e-3), so H is sent as plain bf16 (no
error-compensation terms) and the output is stored as fp16 — the
kernel is DMA-bound and fp16 halves the dominant store traffic. If W
is not exactly bf16-representable, a compensating hi@W_lo term is
added. Per 512-wide output chunk only the 128-row halves of W that
are nonzero are contracted (full-K slices keep every matmul at PE
tile_position (0,0) — mixing sub-128 tile_positions across
accumulation groups crashes the device).

Layout choices, all serving the DMA/drain pipeline:
- H arrives pre-transposed from the host as hT [2*P, BS] bf16
  (real rows then imag rows), so the PE does no transposes and the
  DVE does no transpose drains; matmul lhsT (stationary) slices are
  direct SBUF views.
- PSUM tiles are [128, 2, 512] f32: the real matmul group fills
  [:, 0, :], imag fills [:, 1, :], and ONE cast per chunk drains both
  to fp16 (PSUM reads run the DVE at 1x regardless of dtype, so fewer
  bigger drains win). Drains alternate DVE/ACT 1:1.
- DRAM out is [BS, 8192] fp16, real block then imag block; the drain's
  3D dst AP writes both blocks in one instruction. Host interleaves
  r/i and upcasts to f32 while unsharding.

Sharding: data-parallel over the batch dim, 2048 rows per core x 8 cores.
"""

import os
import sys

if os.path.isdir("/opt/trn_rl_repo") and "/opt/trn_rl_repo" not in sys.path:
    sys.path.insert(0, "/opt/trn_rl_repo")

import ml_dtypes
import numpy as np

_BF16 = np.dtype(ml_dtypes.bfloat16)

_B, _P, _NFFT = 16384, 256, 4096
_NC = 8
_BS = _B // _NC          # rows per core
_PT = 128                # partition tile (batch rows per tile)
_NBT = _BS // _PT        # batch tiles per core
_CH = 512                # output-chunk width (one PSUM bank of fp32)
_NCHUNK = _NFFT // _CH

_cache = {}


def _interp_matrix(pilot_loc, alpha, beta):
    """W [256, 4096] f32 such that out = H @ W reproduces the reference."""
    p = pilot_loc.astype(np.float64) - 1.0  # reference: 1-based -> 0-based
    pp = np.concatenate([p, [float(_NFFT - 1)]])
    t = np.arange(_NFFT)
    left = np.clip(np.searchsorted(pp, t, side="right") - 1, 0, _P - 1)
    right = left + 1
    Wf = np.zeros((_P + 1, _NFFT), np.float64)
    Wf[left, t] += beta.astype(np.float64)
    Wf[right, t] += alpha.astype(np.float64)
    # Hp[:, P] = H[:, P-1] + slope * (NFFT-1 - p[-1]),
    # slope = (H[:, P-1] - H[:, P-2]) / (p[-1] - p[-2])  -> linear in H.
    d = (float(_NFFT - 1) - p[-1]) / (p[-1] - p[-2])
    W = Wf[:_P]
    W[_P - 1] += (1.0 + d) * Wf[_P]
    W[_P - 2] += (-d) * Wf[_P]
    return np.ascontiguousarray(W.astype(np.float32))


def _chunk_pieces(W):
    """Per 512-col chunk: which 128-row halves of W have any nonzeros."""
    out = []
    for c in range(_NCHUNK):
        cols = W[:, c * _CH:(c + 1) * _CH]
        nz = np.nonzero(np.any(cols != 0.0, axis=1))[0]
        k_lo, k_hi = int(nz.min()), int(nz.max())
        pieces = []
        for half in (0, 1):
            if k_lo <= 128 * half + 127 and k_hi >= 128 * half:
                pieces.append(half)
        out.append(tuple(pieces))
    return tuple(out)


def _bf16_split(x):
    hi = x.astype(_BF16)
    lo = (x - hi.astype(np.float32)).astype(_BF16)
    return hi, lo


def _build_program(pieces_per_chunk, use_wlo, store_every=4,
                   copy_cycle="vs"):
    from contextlib import ExitStack

    import concourse.bacc as bacc
    import concourse.bass as bass
    import concourse.mybir as mybir
    import concourse.tile as tile

    f32 = mybir.dt.float32
    f16 = mybir.dt.float16
    bf16 = mybir.dt.bfloat16

    nc = bacc.Bacc("TRN2", target_bir_lowering=False, debug=False,
                   num_devices=_NC)
    # Pre-transposed input: rows [hr^T (256) | hi^T (256)], cols = batch.
    ht_in = nc.dram_tensor("ht", [4 * 128, _BS], bf16,
                           kind="ExternalInput").ap()
    w_in = {"h": nc.dram_tensor("wh", [_P, _NFFT], bf16,
                                kind="ExternalInput").ap()}
    if use_wlo:
        w_in["l"] = nc.dram_tensor("wl", [_P, _NFFT], bf16,
                                   kind="ExternalInput").ap()
    # real block then imag block; host interleaves + upcasts.
    out = nc.dram_tensor("out", [_BS, 2 * _NFFT], f16,
                         kind="ExternalOutput").ap()

    with tile.TileContext(nc) as tc, ExitStack() as ctx:
        const_pool = ctx.enter_context(tc.tile_pool(name="const", bufs=1))
        out_pool = ctx.enter_context(tc.tile_pool(name="outp", bufs=3))
        ps_mm = ctx.enter_context(tc.tile_pool(name="psm", bufs=4,
                                               space="PSUM"))

        # hT SBUF tiles: (x, half) -> [128, BS]; the load order below is
        # chosen so the first chunks' operands land first: the sync ring
        # is idle until the first store, so it carries the h0 hT tiles
        # while the scalar ring streams W (in column halves) and the h1
        # tiles.
        hT = {}
        for x in ("r", "i"):
            for h in (0, 1):
                hT[(x, h)] = const_pool.tile([128, _BS], bf16,
                                             tag=f"hT{x}{h}",
                                             name=f"hT{x}{h}")
        w_sb = {}
        for part in w_in:
            for h in (0, 1):
                w_sb[(part, h)] = const_pool.tile([128, _NFFT], bf16,
                                                  tag=f"w{part}{h}",
                                                  name=f"w{part}{h}")
        nc.sync.dma_start(hT[("r", 0)][:], ht_in[0:128, :])
        nc.sync.dma_start(hT[("i", 0)][:], ht_in[256:384, :])
        half_w = _NFFT // 2
        nc.scalar.dma_start(w_sb[("h", 0)][:, 0:half_w],
                            w_in["h"][0:128, 0:half_w])
        nc.scalar.dma_start(w_sb[("h", 0)][:, half_w:],
                            w_in["h"][0:128, half_w:])
        nc.scalar.dma_start(hT[("r", 1)][:], ht_in[128:256, :])
        nc.scalar.dma_start(hT[("i", 1)][:], ht_in[384:512, :])
        nc.scalar.dma_start(w_sb[("h", 1)][:, 0:half_w],
                            w_in["h"][128:256, 0:half_w])
        nc.scalar.dma_start(w_sb[("h", 1)][:, half_w:],
                            w_in["h"][128:256, half_w:])
        if use_wlo:
            for h in (0, 1):
                nc.scalar.dma_start(
                    w_sb[("l", h)][:], w_in["l"][128 * h:128 * (h + 1), :])

        terms = [("h", "h")]
        if use_wlo:
            terms.append(("h", "l"))

        copy_idx = 0
        for bt in range(_NBT):
            bsl = slice(128 * bt, 128 * (bt + 1))
            ot = out_pool.tile([128, 2, _NFFT], f16, tag="ot")
            for c in range(_NCHUNK):
                pieces = pieces_per_chunk[c]
                n_mm = len(pieces) * len(terms)
                ps = ps_mm.tile([128, 2, _CH], f32, tag="ps")
                for xi, x in enumerate(("r", "i")):
                    j = 0
                    for h in pieces:
                        for (hp, wp) in terms:
                            nc.tensor.matmul(
                                ps[:, xi, :],
                                hT[(x, h)][:, bsl],
                                w_sb[(wp, h)][:, c * _CH:(c + 1) * _CH],
                                start=(j == 0),
                                stop=(j == n_mm - 1),
                            )
                            j += 1
                # one drain for the r+i pair; PSUM reads run DVE/ACT at
                # 1x, so fewer bigger casts win. Alternate engines 1:1.
                dst = ot[:, :, _CH * c:_CH * (c + 1)]
                eng = copy_cycle[copy_idx % len(copy_cycle)]
                if eng == "s":
                    nc.scalar.copy(dst, ps[:])
                else:
                    nc.vector.tensor_copy(dst, ps[:])
                copy_idx += 1
                if (c + 1) % store_every == 0:
                    # store finished slices early; keeps the write ring
                    # fed and shrinks the tail drain.
                    w0 = _CH * (c + 1 - store_every)
                    w1 = _CH * (c + 1)
                    for parity in (0, 1):
                        nc.sync.dma_start(
                            out[bass.ts(bt, 128),
                                _NFFT * parity + w0:_NFFT * parity + w1],
                            ot[:, parity, w0:w1])

    nc.compile()
    return nc


def _get_program(pieces, use_wlo):
    # experiment knobs (default values are the tuned ones)
    se = int(os.environ.get("K_STORE_EVERY", "4"))
    cc = os.environ.get("K_COPY_CYCLE", "vs")
    key = (pieces, use_wlo, se, cc)
    prog = _cache.get(key)
    if prog is None:
        prog = _build_program(pieces, use_wlo, store_every=se,
                              copy_cycle=cc)
        _cache[key] = prog
    return prog


def _make_in_maps(H_real, H_imag, W):
    w_hi, w_lo = _bf16_split(W)
    use_wlo = bool(np.any(np.asarray(w_lo) != 0))
    in_maps = []
    for i in range(_NC):
        sl = slice(i * _BS, (i + 1) * _BS)
        ht = np.ascontiguousarray(np.concatenate(
            [H_real[sl].astype(_BF16).T, H_imag[sl].astype(_BF16).T],
            axis=0))
        m = {"ht": ht, "wh": w_hi}
        if use_wlo:
            m["wl"] = w_lo
        in_maps.append(m)
    return in_maps, use_wlo


def kernel(H_real, H_imag, pilot_loc, alpha, beta):
    H_real = np.ascontiguousarray(np.asarray(H_real, dtype=np.float32))
    H_imag = np.ascontiguousarray(np.asarray(H_imag, dtype=np.float32))
    pilot_loc = np.asarray(pilot_loc, dtype=np.float32)
    alpha = np.asarray(alpha, dtype=np.float32)
    beta = np.asarray(beta, dtype=np.float32)

    W = _interp_matrix(pilot_loc, alpha, beta)
    in_maps, use_wlo = _make_in_maps(H_real, H_imag, W)
    nc = _get_program(_chunk_pieces(W), use_wlo)

    from concourse.bass_utils import run_bass_kernel_spmd

    res = run_bass_kernel_spmd(nc, in_maps, list(range(_NC))).results
    full = np.empty((_B, _NFFT, 2), dtype=np.float32)
    for i, r in enumerate(res):
        o = r["out"]
        full[i * _BS:(i + 1) * _BS, :, 0] = o[:, :_NFFT]
        full[i * _BS:(i + 1) * _BS, :, 1] = o[:, _NFFT:]
    return full
